# revision 10
# baseline (speedup 1.0000x reference)
"""DilateAttention3D (3x3x3 window, dil=1) Trainium2 Bass kernel, 8-core SPMD.

Sharding: core = (b, dc) for b in {0,1}, dc in {0..3}: one batch element and a
D-chunk of 4 (halo 1 from zero-padded k/v) per core.

Per-core tile = (dz, y, xh): 6 heads x 16 queries = 96 voxels, key union
F = 3*3*18 = 162 positions (2 x'-half boxes of 81).

v4 (vs v3 baseline):
 - No on-chip gathers: the QK stationary operand is a strided AP directly
   into a resident k-slab. The slab duplicates x into two 18-wide halves
   [PC, 6, 34, 2, 18] so window APs never straddle, letting the constant
   mask rows (96..112, broadcast over d/y/xh) ride in the same AP.
 - The out-of-window additive mask uses an EXACT integer rank-17
   factorization  M = 1*1^T - I@band : qblk row 96 = 1, rows 97+qi =
   one-hot(qi); km row 96 = -BIG, rows 97+qi = +BIG*band[qi]. All entries
   are fp16-exact, so everything (q, k, v, mask) runs in fp16 at 1 cyc/col
   on the PE (fp32 was 4).
 - TB=16 tiles per DMA group; exp batched 2 tiles per ACT instruction
   (both tiles' score chunks share one PSUM bank).
 - AV output pa [96(h,q), 97(h',c | denom)] written raw (fp16) to DRAM;
   diagonal head-block extraction + 1/denom normalization on host.

Per tile: PE 2 QK + 2 AV matmuls | ACT 1/2 exp | DVE 1/2 copy | no Pool.
"""
import os
import numpy as np

F16 = np.float16
B, d, D, H, W = 2, 96, 16, 32, 32
NH, HD = 6, 16
DL, DLH = 4, 6
NT = DL * H * 2        # 256 tiles/core
BIG = 200.0
SHIFT = 20.0
TB = 16
NG = NT // TB          # 16 groups
R = 17
PC = 96 + R            # 113 contraction rows

_cache = {}


def _build_nc():
    from concourse import bacc, mybir
    import concourse.tile as tile
    from contextlib import ExitStack

    f16 = mybir.dt.float16
    f32 = mybir.dt.float32
    nc = bacc.Bacc(None, target_bir_lowering=False, debug=True)

    qblk_d = nc.declare_dram_parameter("qblk", [NG, PC, TB, 96], f16, isOutput=False)
    km_d = nc.declare_dram_parameter("km", [PC, DL, H, 2, 2, 81], f16, isOutput=False)
    vt_d = nc.declare_dram_parameter("vt", [NG, 81, TB, 2, 97], f16, isOutput=False)
    out_d = nc.declare_dram_parameter("out", [NG, 96, TB, 97], f16, isOutput=True)

    with ExitStack() as ctx:
        tc = ctx.enter_context(tile.TileContext(nc))
        cpool = ctx.enter_context(tc.tile_pool(name="consts", bufs=1))
        qpool = ctx.enter_context(tc.tile_pool(name="q", bufs=3))
        vpool = ctx.enter_context(tc.tile_pool(name="vt", bufs=3))
        epool = ctx.enter_context(tc.tile_pool(name="es", bufs=4))
        opool = ctx.enter_context(tc.tile_pool(name="o", bufs=3))
        pspool = ctx.enter_context(tc.tile_pool(name="ps", bufs=3, space="PSUM"))
        papool = ctx.enter_context(tc.tile_pool(name="pa", bufs=3, space="PSUM"))

        km_sb = cpool.tile([PC, DL, H, 2, 2, 81], f16)
        for dz_ in range(DL):
            nc.sync.dma_start(km_sb[:, dz_], km_d[:, dz_])

        for g_ in range(NG):
            qb = qpool.tile([PC, TB, 96], f16, tag="qb")
            nc.sync.dma_start(qb[:], qblk_d[g_])
            vt = vpool.tile([81, TB, 2, 97], f16, tag="vt")
            nc.sync.dma_start(vt[:], vt_d[g_])
            ob = opool.tile([96, TB, 97], f16, tag="ob")

            ps = None
            pa = None
            for i in range(TB):
                t = TB * g_ + i
                dz, rem = divmod(t, H * 2)
                y, xh = divmod(rem, 2)
                j = i % 2

                if j == 0:
                    ps = pspool.tile([81, 2, 2, 96], f32, tag="ps")
                for c in range(2):
                    nc.tensor.matmul(
                        ps[:, j, c, :],
                        lhsT=km_sb[:, dz, y, xh, c, :],
                        rhs=qb[:, i, :], start=True, stop=True,
                    )
                if j == 1:
                    amt = epool.tile([81, 2, 2, 96], f16, tag="amt")
                    nc.scalar.activation(
                        amt[:], ps[:], mybir.ActivationFunctionType.Exp,
                        scale=0.25,
                    )
                    pa = papool.tile([96, 2, 97], f32, tag="pa")
                    for jj in range(2):
                        for c in range(2):
                            nc.tensor.matmul(
                                pa[:, jj, :],
                                lhsT=amt[:, jj, c, :],
                                rhs=vt[:, i - 1 + jj, c, :],
                                start=(c == 0), stop=(c == 1),
                            )
                    nc.vector.tensor_copy(ob[:, i - 1:i + 1, :], pa[:])
            nc.sync.dma_start(out_d[g_], ob[:])
    nc.compile()
    return nc


def _band():
    band = np.zeros((16, 18), np.float32)
    for qi in range(16):
        band[qi, qi:qi + 3] = 1.0
    return band


def _host_prep(q, k, v, b, dc):
    kp = np.pad(k[b], ((0, 0), (1, 1), (1, 1), (1, 1)))
    vp = np.pad(v[b], ((0, 0), (1, 1), (1, 1), (1, 1)))

    # km slab [PC, DL, H, 2(xh), 2(c), 81]: fully unfolded k windows so the
    # QK stationary is a contiguous [PC, 81] slice. Rows 0..95 = k windows;
    # row 96 = -BIG; rows 97+qi = +BIG * band[qi] (exact rank-17 mask).
    km = np.zeros((PC, DL, H, 2, 2, 81), np.float32)
    ks = kp[:, 4 * dc:4 * dc + DLH]                       # [96,6,34,34]
    swk = np.lib.stride_tricks.sliding_window_view(
        ks, (3, 3, 18), axis=(1, 2, 3))                   # [96,4,32,17,3,3,18]
    swk = swk[:, :, :, ::16]                              # [96,4,32,2,3,3,18]
    km[:96, ..., 0, :] = swk[..., 0:9].reshape(96, DL, H, 2, 81)
    km[:96, ..., 1, :] = swk[..., 9:18].reshape(96, DL, H, 2, 81)
    band = _band()                                        # [16,18]
    bw = np.stack([band[:, 0:9], band[:, 9:18]], axis=1)  # [16,2,9]
    bw = np.broadcast_to(bw[:, None, None, None, :, None, None, :],
                         (16, DL, H, 2, 2, 3, 3, 9))
    # Row 96 carries an extra -SHIFT on every logit (global softmax shift,
    # cancelled by the host-side denominator divide) so exp stays in fp16
    # range: max raw score ~45 -> exp(45/4 - 5) ~ 545 << 65504.
    km[96] = -(BIG + SHIFT)
    km[97:] = BIG * bw.reshape(16, DL, H, 2, 2, 81)

    # qblk [NG, PC, TB, 96]: block-diagonal q + mask left factor.
    qr = q[b].reshape(NH, HD, D, H, W)[:, :, 4 * dc:4 * dc + DL]
    qr = qr.reshape(NH, HD, DL, H, 2, 16)
    qblk = np.zeros((DL, H, 2, PC, 96), np.float32)
    for h in range(NH):
        qblk[:, :, :, 16 * h:16 * h + 16, 16 * h:16 * h + 16] = \
            qr[h].transpose(1, 2, 3, 0, 4)
    qblk[:, :, :, 96, :] = 1.0
    qblk[:, :, :, 97:, :] = np.tile(np.eye(16, dtype=np.float32), (1, NH))
    qblk = qblk.reshape(NG, TB, PC, 96).transpose(0, 2, 1, 3)   # [NG,PC,TB,96]

    # vt [NG, 81, TB, 2, 97]: v windows (key order dz',y',xl) + ones col.
    v_slab = vp[:, 4 * dc:4 * dc + DLH]
    swv = np.lib.stride_tricks.sliding_window_view(
        v_slab, (3, 3, 18), axis=(1, 2, 3))
    wv = swv[:, :, :, ::16].transpose(1, 2, 3, 0, 4, 5, 6)      # [DL,H,2,96,3,3,18]
    vt = np.ones((DL, H, 2, 2, 81, 97), np.float32)
    wvt = wv.transpose(0, 1, 2, 4, 5, 6, 3)
    vt[..., 0, :, :96] = wvt[..., 0:9, :].reshape(DL, H, 2, 81, 96)
    vt[..., 1, :, :96] = wvt[..., 9:18, :].reshape(DL, H, 2, 81, 96)
    vt = vt.transpose(0, 1, 2, 4, 3, 5)                         # [DL,H,2,81,2,97]
    vt = vt.reshape(NG, TB, 81, 2, 97).transpose(0, 2, 1, 3, 4)  # [NG,81,TB,2,97]
    return (np.ascontiguousarray(qblk.astype(F16)),
            np.ascontiguousarray(km.astype(F16)),
            np.ascontiguousarray(vt.astype(F16)))


def kernel(q, k, v):
    q = np.asarray(q, np.float32)
    k = np.asarray(k, np.float32)
    v = np.asarray(v, np.float32)

    if "nc" not in _cache:
        _cache["nc"] = _build_nc()
    nc = _cache["nc"]

    from concourse.bass_utils import run_bass_kernel_spmd

    in_maps = []
    for core in range(8):
        b, dc = divmod(core, 4)
        qblk, km, vt = _host_prep(q, k, v, b, dc)
        in_maps.append({"qblk": qblk, "km": km, "vt": vt})

    res = run_bass_kernel_spmd(nc, in_maps, list(range(8)),
                               trace=bool(int(os.environ.get("KTRACE", "0"))))
    _cache["last_results"] = res

    hsel = np.arange(NH)
    full = np.zeros((B, D, H, W, d), np.float32)
    for core in range(8):
        b, dc = divmod(core, 4)
        ob = res.results[core]["out"].astype(np.float32)   # [NG, 96, TB, 97]
        pa = ob.transpose(0, 2, 1, 3).reshape(NT, 96, 97)
        den = pa[:, :, 96].reshape(NT, NH, 16)
        blocks = pa[:, :, :96].reshape(NT, NH, 16, NH, 16)
        o = blocks[:, hsel, :, hsel, :]                    # [NH, NT, 16, 16]
        o = o.transpose(1, 0, 2, 3) / den[:, :, :, None]   # [NT, NH, 16q, 16c]
        o = o.reshape(DL, H, 2, NH, 16, 16).transpose(0, 1, 2, 4, 3, 5)
        full[b, 4 * dc:4 * dc + DL] = o.reshape(DL, H, W, d)
    return full


# revision 16
# speedup vs baseline: 4.3192x; 4.3192x over previous
"""DilateAttention3D (3x3x3 window, dil=1) Trainium2 Bass kernel, 8-core SPMD.

Sharding: core = (b, dc) for b in {0,1}, dc in {0..3}: one batch element and a
D-chunk of 4 (halo 1 from zero-padded k/v) per core.

Per-core tile = (dz, y, xh): 6 heads x 16 queries = 96 voxels, key union
F = 3*3*18 = 162 positions (2 x'-half boxes of 81).

v4 (vs v3 baseline):
 - No on-chip gathers: the QK stationary operand is a strided AP directly
   into a resident k-slab. The slab duplicates x into two 18-wide halves
   [PC, 6, 34, 2, 18] so window APs never straddle, letting the constant
   mask rows (96..112, broadcast over d/y/xh) ride in the same AP.
 - The out-of-window additive mask uses an EXACT integer rank-17
   factorization  M = 1*1^T - I@band : qblk row 96 = 1, rows 97+qi =
   one-hot(qi); km row 96 = -BIG, rows 97+qi = +BIG*band[qi]. All entries
   are fp16-exact, so everything (q, k, v, mask) runs in fp16 at 1 cyc/col
   on the PE (fp32 was 4).
 - TB=16 tiles per DMA group; exp batched 2 tiles per ACT instruction
   (both tiles' score chunks share one PSUM bank).
 - AV output pa [96(h,q), 97(h',c | denom)] written raw (fp16) to DRAM;
   diagonal head-block extraction + 1/denom normalization on host.

Per tile: PE 2 QK + 2 AV matmuls | ACT 1/2 exp | DVE 1/2 copy | no Pool.
"""
import os
import numpy as np

F16 = np.float16
B, d, D, H, W = 2, 96, 16, 32, 32
NH, HD = 6, 16
DL, DLH = 4, 6
NT = DL * H * 2        # 256 tiles/core
BIG = 200.0
SHIFT = 20.0
TB = 16
NG = NT // TB          # 16 groups
R = 17
PC = 96 + R            # 113 contraction rows

_cache = {}


def _build_nc():
    from concourse import bacc, mybir
    import concourse.tile as tile
    from contextlib import ExitStack

    f16 = mybir.dt.float16
    f32 = mybir.dt.float32
    nc = bacc.Bacc(None, target_bir_lowering=False, debug=True)

    # Partition counts are chosen so DMA descriptors spread across engines
    # (the DGE spreads a transfer over the largest divisor of the partition
    # count <= 16): 96 and 128 -> 16-way, 48 -> 16-way, 33 -> 11-way. The
    # prime 113 would serialize everything on one DMA engine.
    qblk_d = nc.declare_dram_parameter("qblk", [NG, 96, TB, 96], f16, isOutput=False)
    maskq_d = nc.declare_dram_parameter("maskq", [32, TB, 96], f16, isOutput=False)
    km_d = nc.declare_dram_parameter("km", [128, DL, H, 2, 2, 81], f16, isOutput=False)
    vt_d = nc.declare_dram_parameter("vt", [NG, 81, TB, 2, 97], f16, isOutput=False)
    out_d = nc.declare_dram_parameter("out", [NG, 96, TB, 97], f16, isOutput=True)

    with ExitStack() as ctx:
        tc = ctx.enter_context(tile.TileContext(nc))
        cpool = ctx.enter_context(tc.tile_pool(name="consts", bufs=1))
        vpool = ctx.enter_context(tc.tile_pool(name="vt", bufs=3))
        epool = ctx.enter_context(tc.tile_pool(name="es", bufs=4))
        opool = ctx.enter_context(tc.tile_pool(name="o", bufs=3))
        pspool = ctx.enter_context(tc.tile_pool(name="ps", bufs=3, space="PSUM"))
        papool = ctx.enter_context(tc.tile_pool(name="pa", bufs=3, space="PSUM"))

        km_sb = cpool.tile([128, DL, H, 2, 2, 81], f16)
        for dz_ in range(DL):
            nc.sync.dma_start(km_sb[:, dz_], km_d[:, dz_])

        # 3 rotating q buffers: rows 0..95 stream per group; rows 96..127
        # (mask one-hots + zero pad) are constant, filled once.
        qb_bufs = []
        for r in range(3):
            qbb = cpool.tile([128, TB, 96], f16, tag=f"qbuf{r}", name=f"qbuf{r}")
            qb_bufs.append(qbb)
            nc.sync.dma_start(qbb[96:128, :, :], maskq_d[:])

        for g_ in range(NG):
            qb = qb_bufs[g_ % 3]
            nc.sync.dma_start(qb[0:96, :, :], qblk_d[g_])
            vt = vpool.tile([81, TB, 2, 97], f16, tag="vt")
            nc.sync.dma_start(vt[0:48], vt_d[g_, 0:48])
            nc.sync.dma_start(vt[48:81], vt_d[g_, 48:81])
            ob = opool.tile([96, TB, 97], f16, tag="ob")

            ps = None
            pa = None
            for i in range(TB):
                t = TB * g_ + i
                dz, rem = divmod(t, H * 2)
                y, xh = divmod(rem, 2)
                j = i % 2

                if j == 0:
                    ps = pspool.tile([81, 2, 2, 96], f32, tag="ps")
                for c in range(2):
                    nc.tensor.matmul(
                        ps[:, j, c, :],
                        lhsT=km_sb[:, dz, y, xh, c, :],
                        rhs=qb[:, i, :], start=True, stop=True,
                    )  # contraction over all 128 partitions; rows 113..127 zero
                if j == 1:
                    amt = epool.tile([81, 2, 2, 96], f16, tag="amt")
                    nc.scalar.activation(
                        amt[:], ps[:], mybir.ActivationFunctionType.Exp,
                        scale=0.25,
                    )
                    pa = papool.tile([96, 2, 97], f32, tag="pa")
                    for jj in range(2):
                        for c in range(2):
                            nc.tensor.matmul(
                                pa[:, jj, :],
                                lhsT=amt[:, jj, c, :],
                                rhs=vt[:, i - 1 + jj, c, :],
                                start=(c == 0), stop=(c == 1),
                            )
                    nc.vector.tensor_copy(ob[:, i - 1:i + 1, :], pa[:])
            nc.sync.dma_start(out_d[g_], ob[:])
    nc.compile()
    return nc


def _band():
    band = np.zeros((16, 18), np.float32)
    for qi in range(16):
        band[qi, qi:qi + 3] = 1.0
    return band


def _host_prep(q, k, v, b, dc):
    kp = np.pad(k[b], ((0, 0), (1, 1), (1, 1), (1, 1)))
    vp = np.pad(v[b], ((0, 0), (1, 1), (1, 1), (1, 1)))

    # km slab [128, DL, H, 2(xh), 2(c), 81]: fully unfolded k windows so the
    # QK stationary is a contiguous [128, 81] slice. Rows 0..95 = k windows;
    # row 96 = -BIG-SHIFT; rows 97+qi = +BIG * band[qi] (exact rank-17
    # mask); rows 113..127 = 0 (pad so DMA spreads 16-way).
    km = np.zeros((128, DL, H, 2, 2, 81), np.float32)
    ks = kp[:, 4 * dc:4 * dc + DLH]                       # [96,6,34,34]
    swk = np.lib.stride_tricks.sliding_window_view(
        ks, (3, 3, 18), axis=(1, 2, 3))                   # [96,4,32,17,3,3,18]
    swk = swk[:, :, :, ::16]                              # [96,4,32,2,3,3,18]
    km[:96, ..., 0, :] = swk[..., 0:9].reshape(96, DL, H, 2, 81)
    km[:96, ..., 1, :] = swk[..., 9:18].reshape(96, DL, H, 2, 81)
    band = _band()                                        # [16,18]
    bw = np.stack([band[:, 0:9], band[:, 9:18]], axis=1)  # [16,2,9]
    bw = np.broadcast_to(bw[:, None, None, None, :, None, None, :],
                         (16, DL, H, 2, 2, 3, 3, 9))
    # Row 96 carries an extra -SHIFT on every logit (global softmax shift,
    # cancelled by the host-side denominator divide) so exp stays in fp16
    # range: max raw score ~45 -> exp(45/4 - 5) ~ 545 << 65504.
    km[96] = -(BIG + SHIFT)
    km[97:113] = BIG * bw.reshape(16, DL, H, 2, 2, 81)

    # qblk [NG, 96, TB, 96]: block-diagonal q only (mask rows 96..127 are
    # constant and live in the resident SBUF q-buffers, see maskq).
    qr = q[b].reshape(NH, HD, D, H, W)[:, :, 4 * dc:4 * dc + DL]
    qr = qr.reshape(NH, HD, DL, H, 2, 16)
    qblk = np.zeros((DL, H, 2, 96, 96), np.float32)
    for h in range(NH):
        qblk[:, :, :, 16 * h:16 * h + 16, 16 * h:16 * h + 16] = \
            qr[h].transpose(1, 2, 3, 0, 4)
    qblk = qblk.reshape(NG, TB, 96, 96).transpose(0, 2, 1, 3)   # [NG,96,TB,96]

    # vt [NG, 81, TB, 2, 97]: v windows (key order dz',y',xl) + ones col.
    v_slab = vp[:, 4 * dc:4 * dc + DLH]
    swv = np.lib.stride_tricks.sliding_window_view(
        v_slab, (3, 3, 18), axis=(1, 2, 3))
    wv = swv[:, :, :, ::16].transpose(1, 2, 3, 0, 4, 5, 6)      # [DL,H,2,96,3,3,18]
    vt = np.ones((DL, H, 2, 2, 81, 97), np.float32)
    wvt = wv.transpose(0, 1, 2, 4, 5, 6, 3)
    vt[..., 0, :, :96] = wvt[..., 0:9, :].reshape(DL, H, 2, 81, 96)
    vt[..., 1, :, :96] = wvt[..., 9:18, :].reshape(DL, H, 2, 81, 96)
    vt = vt.transpose(0, 1, 2, 4, 3, 5)                         # [DL,H,2,81,2,97]
    vt = vt.reshape(NG, TB, 81, 2, 97).transpose(0, 2, 1, 3, 4)  # [NG,81,TB,2,97]
    return (np.ascontiguousarray(qblk.astype(F16)),
            np.ascontiguousarray(km.astype(F16)),
            np.ascontiguousarray(vt.astype(F16)))


def _maskq():
    # Constant rows 96..127 of the q operand: row 0 (=96) all-ones, rows
    # 1..16 (=97+qi) one-hot per query x-position, rows 17..31 zero pad.
    mq = np.zeros((32, TB, 96), np.float32)
    mq[0] = 1.0
    mq[1:17] = np.tile(np.eye(16, dtype=np.float32), (1, NH))[:, None, :]
    return mq.astype(F16)


def kernel(q, k, v):
    q = np.asarray(q, np.float32)
    k = np.asarray(k, np.float32)
    v = np.asarray(v, np.float32)

    if "nc" not in _cache:
        _cache["nc"] = _build_nc()
    nc = _cache["nc"]

    from concourse.bass_utils import run_bass_kernel_spmd

    maskq = _maskq()
    in_maps = []
    for core in range(8):
        b, dc = divmod(core, 4)
        qblk, km, vt = _host_prep(q, k, v, b, dc)
        in_maps.append({"qblk": qblk, "km": km, "vt": vt, "maskq": maskq})

    res = run_bass_kernel_spmd(nc, in_maps, list(range(8)),
                               trace=bool(int(os.environ.get("KTRACE", "0"))))
    _cache["last_results"] = res

    hsel = np.arange(NH)
    full = np.zeros((B, D, H, W, d), np.float32)
    for core in range(8):
        b, dc = divmod(core, 4)
        ob = res.results[core]["out"].astype(np.float32)   # [NG, 96, TB, 97]
        pa = ob.transpose(0, 2, 1, 3).reshape(NT, 96, 97)
        den = pa[:, :, 96].reshape(NT, NH, 16)
        blocks = pa[:, :, :96].reshape(NT, NH, 16, NH, 16)
        o = blocks[:, hsel, :, hsel, :]                    # [NH, NT, 16, 16]
        o = o.transpose(1, 0, 2, 3) / den[:, :, :, None]   # [NT, NH, 16q, 16c]
        o = o.reshape(DL, H, 2, NH, 16, 16).transpose(0, 1, 2, 4, 3, 5)
        full[b, 4 * dc:4 * dc + DL] = o.reshape(DL, H, W, d)
    return full
